# revision 1
# baseline (speedup 1.0000x reference)
"""Fused QKV projection + RMSNorm + RoPE + GQA repeat for Trainium2.

Reference computation (per nn_Attention_33681133535344):
    q = rope(rmsnorm(x @ Wq, gq))   -> (B, H, T, DH)
    k = rope(rmsnorm(x @ Wk, gk))   -> repeat -> (B, H, T, DH)
    v = x @ Wv                      -> repeat -> (B, H, T, DH)

Sharding: rows of flattened (B*T, D) x are split across the 8 NeuronCores
(1024 tokens each); weights are replicated. RMSNorm reduces over the full
feature dim, which is row-local under this sharding, so no collectives are
needed. Each core computes x_shard @ [Wq|Wk|Wv] as one 1024x4096x6144
bf16 matmul stream (f32 PSUM accumulation), applies RoPE at PSUM eviction
(RoPE commutes with the per-token RMS scale), accumulates sum-of-squares
from pre-rope PSUM via an ACT Square with row-sum accumulator, stages
roped-unnormalized
q/k to DRAM, and applies scale*gamma in a fused second pass that overlaps
the tail of the matmul stream. The GQA head-repeat is pure duplication and
is done on the host during unsharding.
"""

import sys

sys.path.insert(0, "/opt/trn_rl_repo")

import numpy as np
import ml_dtypes

B, T, D = 2, 4096, 4096
H, HKV = 32, 8
DH = D // H  # 128
EPS = 1e-5
ROPE_BASE = 10000.0

NCORES = 8
P = 128
TLOC = (B * T) // NCORES  # 1024 tokens per core
TT = TLOC // P  # 8 token tiles per core
KO = D // P  # 32 contraction chunks
NQ = D  # 4096 q cols
NKV = HKV * DH  # 1024 k cols (same for v)
NCOLS = NQ + 2 * NKV  # 6144 fused output cols
NT = 512  # slab width == matmul moving free dim
NSLAB = NCOLS // NT  # 12 (8 q, 2 k, 2 v)
Q_SLABS = NQ // NT  # 8
K_SLABS = NKV // NT  # 2
PH2_CH = 1024  # phase-2 chunk width

BF16 = ml_dtypes.bfloat16

_CACHE = {}


def _build():
    import concourse.mybir as mybir
    import concourse.tile as tile
    from concourse import bacc

    f32 = mybir.dt.float32
    bf16 = mybir.dt.bfloat16
    mult = mybir.AluOpType.mult

    nc = bacc.Bacc("TRN2", target_bir_lowering=False, debug=False)

    # layouts chosen so every DMA is contiguous per partition row:
    # xt[tt, ki, ko, t], w[oc, ki, ko, n] (slab-major)
    xt = nc.declare_dram_parameter("xt", [TT, P, KO, P], bf16, isOutput=False)
    w = nc.declare_dram_parameter("w", [NSLAB, P, KO, NT], bf16, isOutput=False)
    cose = nc.declare_dram_parameter("cose", [P, TT, DH], f32, isOutput=False)
    sine = nc.declare_dram_parameter("sine", [P, TT, DH], f32, isOutput=False)
    grep = nc.declare_dram_parameter("grep", [P, NQ + NKV], f32, isOutput=False)
    q_out = nc.declare_dram_parameter("q", [TT, P, NQ], f32, isOutput=True)
    k_out = nc.declare_dram_parameter("k", [TT, P, NKV], f32, isOutput=True)
    v_out = nc.declare_dram_parameter("v", [TT, P, NKV], f32, isOutput=True)

    NH = NT // DH  # heads per slab (4)

    with tile.TileContext(nc) as tc:
        with (
            tc.tile_pool(name="const", bufs=1) as const,
            tc.tile_pool(name="xp", bufs=1) as xp,
            tc.tile_pool(name="wp", bufs=2) as wp,
            tc.tile_pool(name="ev", bufs=2) as ev,
            tc.tile_pool(name="ph2", bufs=4) as ph2,
            tc.tile_pool(name="psp", bufs=8, space="PSUM") as psp,
            tc.tile_pool(name="dram", bufs=1, space="DRAM") as dram,
        ):
            # first W quarter + first x chunk lead the DMA queue so the PE
            # can start streaming within a few microseconds
            KQ = 4  # k-quarters per W slab
            KOQ = KO // KQ
            wq_tiles = {}

            def load_wq(oc, q):
                t = wp.tile([P, KOQ, NT], bf16, tag=f"wq{q}")
                nc.sync.dma_start(t[:], w[oc, :, q * KOQ : (q + 1) * KOQ, :])
                return t

            def load_wslab(oc):
                wq_tiles[oc] = [load_wq(oc, q) for q in range(KQ)]

            # x tiles split into ko-halves so the first matmul group's
            # dependency is only 512 KB; W0-q0 + x0-half0 are issued first
            KOH = KO // 2
            xsb_t = [
                [
                    xp.tile([P, KOH, P], bf16, tag=f"x{tt}h{h}", name=f"xsb{tt}h{h}")
                    for h in range(2)
                ]
                for tt in range(TT)
            ]

            def load_x(tt, h):
                nc.sync.dma_start(
                    xsb_t[tt][h][:], xt[tt, :, h * KOH : (h + 1) * KOH, :]
                )

            w0q0 = load_wq(0, 0)
            load_x(0, 0)
            wq_tiles[0] = [w0q0] + [load_wq(0, q) for q in range(1, KQ)]
            load_x(0, 1)
            for tt in range(1, TT):
                load_x(tt, 0)
                load_x(tt, 1)
            cosb = const.tile([P, TT, DH], f32)
            nc.sync.dma_start(cosb[:], cose[:])
            sinb = const.tile([P, TT, DH], f32)
            nc.sync.dma_start(sinb[:], sine[:])
            gsb = const.tile([P, NQ + NKV], f32)
            nc.sync.dma_start(gsb[:], grep[:])

            epsb = const.tile([P, 1], f32)
            nc.vector.memset(epsb[:], EPS)
            # HAM warm-up: ~12 matmuls on uninitialized SBUF garbage during
            # the initial input-DMA window (PE is idle 7-16us otherwise).
            # ~4.5us of PE activity flips the clock gate to 2.4 GHz before
            # the real stream starts; the dummy PSUM tile is never read.
            warm_l = const.tile([P, P], bf16)
            nc.vector.memset(warm_l[:], 0.0)
            warm_r = const.tile([P, NT], bf16)
            nc.vector.memset(warm_r[:], 0.0)
            warm_ps = psp.tile([P, NT], f32, tag="ps")
            for i in range(32):
                nc.tensor.matmul(
                    warm_ps[:], warm_l[:], warm_r[:], start=True, stop=True
                )

            statq = const.tile([P, TT], f32)
            nc.vector.memset(statq[:], 0.0)
            statk = const.tile([P, TT], f32)
            nc.vector.memset(statk[:], 0.0)
            scaleq = const.tile([P, TT], f32)
            scalek = const.tile([P, TT], f32)

            qs = dram.tile([TT, P, NQ], f32)
            ks = dram.tile([TT, P, NKV], f32)

            def do_slab(oc, fillers=None):
                col0 = oc * NT
                if oc not in wq_tiles:
                    load_wslab(oc)
                wsb = wq_tiles.pop(oc)
                if oc + 1 < NSLAB:
                    load_wslab(oc + 1)  # prefetch next slab
                for tt in range(TT):
                    ps = psp.tile([P, NT], f32, tag="ps")
                    for ko in range(KO):
                        nc.tensor.matmul(
                            ps[:],
                            xsb_t[tt][ko // KOH][:, ko % KOH, :],
                            wsb[ko // KOQ][:, ko % KOQ, :],
                            start=(ko == 0),
                            stop=(ko == KO - 1),
                        )
                    if col0 < NQ + NKV:
                        is_q = col0 < NQ
                        stats = statq if is_q else statk
                        stage = qs if is_q else ks
                        scol = col0 if is_q else col0 - NQ
                        # RoPE: out = ps * cosE + swap_pairs(ps) * sinE
                        # (sinE carries the -sin on even lanes)
                        ps4 = ps[:].rearrange("p (h j s) -> p h j s", h=NH, s=2)
                        rot = ev.tile([P, NT], f32, tag="rot", bufs=3)
                        rot4 = rot[:].rearrange("p (h j s) -> p h j s", h=NH, s=2)
                        nc.scalar.copy(rot4[:, :, :, 0], ps4[:, :, :, 1])
                        nc.scalar.copy(rot4[:, :, :, 1], ps4[:, :, :, 0])
                        cos_bc = cosb[:, tt, None, :].to_broadcast((P, NH, DH))
                        sin_bc = sinb[:, tt, None, :].to_broadcast((P, NH, DH))
                        st = ev.tile([P, NT], f32, tag="st", bufs=3)
                        st3 = st[:].rearrange("p (h d) -> p h d", h=NH)
                        ps3 = ps[:].rearrange("p (h d) -> p h d", h=NH)
                        rot3 = rot[:].rearrange("p (h d) -> p h d", h=NH)
                        nc.vector.tensor_tensor(st3, ps3, cos_bc, mult)
                        nc.vector.tensor_tensor(rot3, rot3, sin_bc, mult)
                        nc.vector.tensor_add(st[:], st[:], rot[:])
                        # per-token sum of squares of the pre-norm projection,
                        # from PSUM via ACT Square (+ per-partition row sum);
                        # tensor_tensor_reduce faults at runtime on this stack
                        sq = ev.tile([P, NT], f32, tag="sq", bufs=1)
                        acc = ev.tile([P, 1], f32, tag="acc")
                        nc.scalar.activation(
                            sq[:],
                            ps[:],
                            mybir.ActivationFunctionType.Square,
                            accum_out=acc[:, 0:1],
                        )
                        nc.vector.tensor_add(
                            stats[:, tt : tt + 1], stats[:, tt : tt + 1], acc[:, 0:1]
                        )
                        nc.sync.dma_start(stage[tt, :, scol : scol + NT], st[:])
                    else:
                        scol = col0 - NQ - NKV
                        vt = ev.tile([P, NT], f32, tag="vt")
                        nc.vector.tensor_copy(vt[:], ps[:])
                        nc.sync.dma_start(v_out[tt, :, scol : scol + NT], vt[:])
                    if fillers:
                        fillers.pop(0)()

            def phase2_scale(stats, scale_tile, nd):
                # scale = 1 / sqrt(ssq/nd + eps)
                nc.scalar.activation(
                    scale_tile[:],
                    stats[:],
                    mybir.ActivationFunctionType.Sqrt,
                    bias=epsb[:, 0:1],
                    scale=1.0 / nd,
                )
                nc.vector.reciprocal(scale_tile[:], scale_tile[:])

            def phase2_chunks(stage, scale_tile, goff, out_ext, tt, c0s):
                # phase-2 DMAs ride the (idle) GpSimd queue so they can't
                # delay W-slab prefetch issues on the Sync queue
                for c0 in c0s:
                    t2 = ph2.tile([P, PH2_CH], f32, tag="p2")
                    nc.gpsimd.dma_start(t2[:], stage[tt, :, c0 : c0 + PH2_CH])
                    nc.vector.scalar_tensor_tensor(
                        out=t2[:],
                        in0=t2[:],
                        scalar=scale_tile[:, tt : tt + 1],
                        in1=gsb[:, goff + c0 : goff + c0 + PH2_CH],
                        op0=mult,
                        op1=mult,
                    )
                    nc.gpsimd.dma_start(out_ext[tt, :, c0 : c0 + PH2_CH], t2[:])

            def p2_filler(stage, scale_tile, goff, out_ext, tt, c0s):
                return lambda: phase2_chunks(stage, scale_tile, goff, out_ext, tt, c0s)

            # slabs 0..7 = q, 8..9 = k, 10..11 = v. Phase-2 (scale*gamma on
            # the staged roped projections) is interleaved one half-token-tile
            # per matmul group across slabs 8..10 so its DVE/DMA load never
            # bursts; slab 11 runs clean to keep the kernel tail short.
            for oc in range(Q_SLABS):
                do_slab(oc)
            phase2_scale(statq, scaleq, NQ)
            qf = [
                p2_filler(qs, scaleq, 0, q_out, tt,
                          range(h * PH2_CH * 2, (h + 1) * PH2_CH * 2, PH2_CH))
                for tt in range(TT)
                for h in range(2)
            ]
            do_slab(Q_SLABS, fillers=qf[:TT])
            do_slab(Q_SLABS + 1, fillers=qf[TT:])
            phase2_scale(statk, scalek, NKV)
            kf = [
                p2_filler(ks, scalek, NQ, k_out, tt, range(0, NKV, PH2_CH))
                for tt in range(TT)
            ]
            do_slab(Q_SLABS + K_SLABS, fillers=kf)
            do_slab(Q_SLABS + K_SLABS + 1)

    nc.compile()
    return nc


def _in_maps(x, Wq, Wk, Wv, gq, gk):
    Wcat = np.concatenate([Wq, Wk, Wv], axis=1)  # (D, NCOLS)
    # [NSLAB, P, KO, NT]: slab-major, contiguous per (slab, partition) row
    w_arr = np.ascontiguousarray(
        Wcat.reshape(KO, P, NSLAB, NT).transpose(2, 1, 0, 3)
    ).astype(BF16)
    g_rep = np.ascontiguousarray(
        np.tile(np.concatenate([gq, gk])[None, :], (P, 1))
    ).astype(np.float32)

    xflat = np.ascontiguousarray(x.reshape(B * T, D))

    inv = 1.0 / (ROPE_BASE ** (np.arange(0, DH, 2, dtype=np.float32) / DH))
    inv = inv.astype(np.float32)

    maps = []
    for c in range(NCORES):
        rows = xflat[c * TLOC : (c + 1) * TLOC]  # (TLOC, D)
        # [TT, P, KO, P]: xt[tt, ki, ko, t] = rows[tt*P + t, ko*P + ki]
        xt = np.ascontiguousarray(
            rows.T.reshape(KO, P, TT, P).transpose(2, 1, 0, 3)
        ).astype(BF16)
        t0 = (c % (T // TLOC)) * TLOC
        t_abs = np.arange(t0, t0 + TLOC, dtype=np.float32)
        ang = t_abs[:, None] * inv[None, :]  # (TLOC, DH/2)
        cos = np.cos(ang).astype(np.float32)
        sin = np.sin(ang).astype(np.float32)
        cosE = np.repeat(cos, 2, axis=1)  # (TLOC, DH)
        sinE = np.stack([-sin, sin], axis=-1).reshape(TLOC, DH)
        cos_arr = np.ascontiguousarray(cosE.reshape(TT, P, DH).transpose(1, 0, 2))
        sin_arr = np.ascontiguousarray(
            sinE.reshape(TT, P, DH).transpose(1, 0, 2)
        ).astype(np.float32)
        maps.append(
            {"xt": xt, "w": w_arr, "cose": cos_arr, "sine": sin_arr, "grep": g_rep}
        )
    return maps


def _assemble(results):
    q = np.empty((B * T, NQ), np.float32)
    k = np.empty((B * T, NKV), np.float32)
    v = np.empty((B * T, NKV), np.float32)
    for c in range(NCORES):
        q[c * TLOC : (c + 1) * TLOC] = results[c]["q"].reshape(TLOC, NQ)
        k[c * TLOC : (c + 1) * TLOC] = results[c]["k"].reshape(TLOC, NKV)
        v[c * TLOC : (c + 1) * TLOC] = results[c]["v"].reshape(TLOC, NKV)
    q = np.ascontiguousarray(q.reshape(B, T, H, DH).transpose(0, 2, 1, 3))
    k = k.reshape(B, T, HKV, DH).transpose(0, 2, 1, 3)
    v = v.reshape(B, T, HKV, DH).transpose(0, 2, 1, 3)
    n_rep = H // HKV
    k = np.repeat(k, n_rep, axis=1)
    v = np.repeat(v, n_rep, axis=1)
    return q, k, v


def run(inputs, trace=False, trace_cores=None):
    from concourse.bass_utils import run_bass_kernel_spmd

    x = np.asarray(inputs["x"], dtype=np.float32)
    Wq = np.asarray(inputs["Wq"], dtype=np.float32)
    Wk = np.asarray(inputs["Wk"], dtype=np.float32)
    Wv = np.asarray(inputs["Wv"], dtype=np.float32)
    gq = np.asarray(inputs["gq"], dtype=np.float32)
    gk = np.asarray(inputs["gk"], dtype=np.float32)

    if "nc" not in _CACHE:
        _CACHE["nc"] = _build()
    nc = _CACHE["nc"]

    maps = _in_maps(x, Wq, Wk, Wv, gq, gk)
    res = run_bass_kernel_spmd(
        nc, maps, core_ids=list(range(NCORES)), trace=trace, trace_cores=trace_cores
    )
    out = _assemble(res.results)
    return out, res


def kernel(**inputs):
    out, _ = run(inputs, trace=False)
    return out

